# revision 25
# baseline (speedup 1.0000x reference)
"""Cost-volume kernel for Trainium2 (Bass/Tile), 8-core SPMD.

volume[n, c, d, h, w] = left[n,c,h,w] * right[n,c,h,w-d]  (0 where w < d)

Sharding: rows (flattened n,c,h = 8704) split as 1088 per core; every core
computes ALL 48 disparities for its rows (shift is along W, so row sharding
needs no halo and inputs are read once).

The kernel is HBM-store bound, so the store stream is minimized two ways:
 - fp16 output (harness gate is rel_err < 2e-2; fp16 product error ~7e-4).
 - packed layout: for disparity d only the ~(W-d) valid products
   packed[d][r, j] = left[r, d+j] * right[r, j] are stored; the host
   scatters them into a zero-filled full volume.

All multiplies run on DVE (gpsimd tensor_tensor measured ~3x slower and the
ACT engine has no two-tensor op). DVE does ~0.52 ns/elem (2x_1p fp16 mode)
plus ~350 ns fixed cost per instruction, so disparities are processed in
GROUPS of 4 per instruction using a 4-D access pattern whose group dim has
stride +1 on the left operand (one extra shift per group member) and
stride 0 (broadcast) on the right operand. Group blocks share a uniform
width W-g, so members i>0 carry (d-g) junk columns that the host ignores.

Main chunk: rows [64,1088) as [128 partitions x 8 rows]; per-partition
lines are DRAM-contiguous so every load/store is a >=3 KB-per-partition
single DMA. Disparities 0..3 are emitted per-d (even width) so the store
stream starts after ~1.4 us; d 4..47 go in 11 groups of 4. The 64-row
tail is ONE flat [64, 48, 240] multiply + one store. Big stores ride the
ACT HWDGE ring; loads and the tail store ride SP.
"""

import os

import numpy as np

import bass_rust
import concourse.bacc as bacc
import concourse.mybir as mybir
from concourse.bass_utils import run_bass_kernel_spmd
from concourse.mybir import AluOpType
from concourse.tile import TileContext

N, C, H, W = 2, 32, 136, 240
MAX_DISP = 48
NCORES = 8
R = N * C * H                   # 8704 rows total
ROWS = R // NCORES              # 1088 rows per core
TAIL = 64                       # leftover rows (1088 = 64 + 128*8)
CPP = 8                         # rows per partition in the main chunk
G = 4                           # disparities per grouped DVE instruction
NSOLO = 4                       # leading disparities emitted per-d
LBW = CPP * W + 8               # lb tile width (pad: group reads to 1919+3)
LTW = W + MAX_DISP              # lt tile width (tail reads to 286)

# Per-d blocks: leading d 0..3 (pipeline ramp) and trailing d 44..47
# (drain taper). Groups of G=4 cover d 4..43.
SOLOS = list(range(NSOLO)) + list(range(MAX_DISP - NSOLO, MAX_DISP))
GROUPS = list(range(NSOLO, MAX_DISP - NSOLO, G))

# Even-rounded block width for the per-d blocks (alignment-safe).
BW = {d: W - d + ((W - d) & 1) for d in SOLOS}

# out_big per-partition column offsets.
XB = {}
_col = 0
for _d in SOLOS:
    XB[_d] = _col
    _col += CPP * BW[_d]
for _g in GROUPS:
    XB[_g] = _col
    _col += G * CPP * (W - _g)
XBTOT = _col

_NC_CACHE = None
LAST_RESULTS = None  # BassKernelResults of the most recent run (for test.py)


def _build_bass():
    # Bacc (not plain Bass): its finalize() runs the compile pipeline incl.
    # generate_event_semaphores, which splits multi-sem waits that walrus
    # rejects ("Too many sync wait commands").
    nc = bacc.Bacc()
    left = nc.dram_tensor("left", [ROWS, W], mybir.dt.float16, kind="ExternalInput")
    right = nc.dram_tensor("right", [ROWS, W], mybir.dt.float16, kind="ExternalInput")
    out_big = nc.dram_tensor(
        "out_big", [128, XBTOT], mybir.dt.float16, kind="ExternalOutput"
    )
    out_tail = nc.dram_tensor(
        "out_tail", [TAIL, MAX_DISP * W], mybir.dt.float16, kind="ExternalOutput"
    )

    with (
        TileContext(nc) as tc,
        tc.tile_pool(name="lpool", bufs=1) as lpool,
        tc.tile_pool(name="rpool", bufs=1) as rpool,
        tc.tile_pool(name="osolo", bufs=4) as osolo,
        tc.tile_pool(name="ogrp", bufs=5) as ogrp,
        tc.tile_pool(name="otail", bufs=1) as otail,
    ):
        lb = lpool.tile([128, LBW], mybir.dt.float16, tag="lbig")
        rb = rpool.tile([128, CPP * W], mybir.dt.float16, tag="rbig")
        lt = lpool.tile([TAIL, LTW], mybir.dt.float16, tag="ltail")
        rt = rpool.tile([TAIL, W], mybir.dt.float16, tag="rtail")

        # Loads split across both HWDGE rings and into q-halves, so the
        # d=0 first-half multiply (which only reads q 0..3) can start after
        # ~1.5 us of load data instead of the full 2.6 us.
        HQ = CPP // 2
        lsrc = left[TAIL:ROWS, :].rearrange("(p q) w -> p q w", p=128)
        rsrc = right[TAIL:ROWS, :].rearrange("(p q) w -> p q w", p=128)
        lbq = lb[:, 0 : CPP * W].rearrange("p (q w) -> p q w", w=W)
        rbq = rb[:].rearrange("p (q w) -> p q w", w=W)
        nc.sync.dma_start(out=lbq[:, 0:HQ, :], in_=lsrc[:, 0:HQ, :])
        nc.scalar.dma_start(out=rbq[:, 0:HQ, :], in_=rsrc[:, 0:HQ, :])
        nc.sync.dma_start(out=lbq[:, HQ:CPP, :], in_=lsrc[:, HQ:CPP, :])
        nc.scalar.dma_start(out=rbq[:, HQ:CPP, :], in_=rsrc[:, HQ:CPP, :])
        nc.sync.dma_start(out=lt[:, 0:W], in_=left[0:TAIL, :])
        nc.scalar.dma_start(out=rt[:], in_=right[0:TAIL, :])

        lbv = lb[:, 0 : CPP * W].rearrange("p (q w) -> p q w", w=W)
        rbv = rb[:].rearrange("p (q w) -> p q w", w=W)
        lb_ap = lb[:]

        # Leading disparities per-d: store stream starts after one ~1 us op.
        # Ring selection for big stores (A/B: COSTVOL_DUALRING=1 alternates
        # between both HWDGE rings).
        if os.environ.get("COSTVOL_DUALRING", "0") == "1":
            ring = [nc.scalar, nc.sync]
            nstores = [0]

            def bigstore(dram_ap, sbuf_ap):
                ring[nstores[0] % 2].dma_start(out=dram_ap, in_=sbuf_ap)
                nstores[0] += 1
        else:

            def bigstore(dram_ap, sbuf_ap):
                nc.scalar.dma_start(out=dram_ap, in_=sbuf_ap)

        def solo_part(d, q0, q1):
            bw = BW[d]
            ob = solo_tiles[d]
            nq = q1 - q0
            in0 = bass_rust.AP(
                lb_ap.tensor,
                lb_ap.offset + q0 * W + d,
                [[LBW, 128], [W, nq], [1, bw]],
            )
            nc.vector.tensor_tensor(
                ob[:, q0 * bw : q1 * bw].rearrange("p (q w) -> p q w", w=bw),
                in0,
                rbv[:, q0:q1, 0:bw],
                AluOpType.mult,
            )
            bigstore(
                out_big[:, XB[d] + q0 * bw : XB[d] + q1 * bw],
                ob[:, q0 * bw : q1 * bw],
            )

        solo_tiles = {}

        def solo(d, halves=False):
            solo_tiles[d] = osolo.tile(
                [128, CPP * W], mybir.dt.float16, name="ob_solo"
            )
            for q0, q1 in ([(0, HQ), (HQ, CPP)] if halves else [(0, CPP)]):
                solo_part(d, q0, q1)

        # d0/d1 run as interleaved q-halves: the first-half multiplies only
        # wait on the first-half loads, covering the second halves' DMA
        # completion latency with useful work.
        solo_tiles[0] = osolo.tile([128, CPP * W], mybir.dt.float16, name="ob_solo")
        solo_tiles[1] = osolo.tile([128, CPP * W], mybir.dt.float16, name="ob_solo")
        solo_part(0, 0, HQ)
        solo_part(1, 0, HQ)
        solo_part(0, HQ, CPP)
        solo_part(1, HQ, CPP)
        for d in range(2, NSOLO):
            solo(d)

        # Grouped disparities: one 4-D instruction per 4 d's. Left operand
        # group dim strides +1 (shift), right operand broadcasts.
        for g in GROUPS:
            wg = W - g
            ob = ogrp.tile([128, G * CPP * (W - NSOLO)], mybir.dt.float16)
            in0 = bass_rust.AP(
                lb_ap.tensor,
                lb_ap.offset + g,
                [[LBW, 128], [1, G], [W, CPP], [1, wg]],
            )
            in1 = rbv[:, :, 0:wg].unsqueeze(1).broadcast_to([128, G, CPP, wg])
            nc.vector.tensor_tensor(
                ob[:, 0 : G * CPP * wg].rearrange(
                    "p (i q w) -> p i q w", i=G, q=CPP
                ),
                in0,
                in1,
                AluOpType.mult,
            )
            bigstore(
                out_big[:, XB[g] : XB[g] + G * CPP * wg],
                ob[:, 0 : G * CPP * wg],
            )
            if g == 2 * G + NSOLO:
                # Tail: one flat [64, 48, 240] multiply + one store. On DVE:
                # a concurrent Pool op stalls DVE for its whole duration
                # (SBUF contention), so Pool is useless here.
                ot = otail.tile([TAIL, MAX_DISP * W], mybir.dt.float16)
                t_in0 = bass_rust.AP(
                    lt[:].tensor,
                    lt[:].offset,
                    [[LTW, TAIL], [1, MAX_DISP], [1, W]],
                )
                t_in1 = rt[:].unsqueeze(1).broadcast_to([TAIL, MAX_DISP, W])
                nc.vector.tensor_tensor(
                    ot[:].rearrange("p (i w) -> p i w", w=W),
                    t_in0,
                    t_in1,
                    AluOpType.mult,
                )
                # Two half stores on SP: one 23 KB-per-partition store runs
                # at half DMA-engine rate; <=16 KB packets run at full rate.
                half = MAX_DISP * W // 2
                nc.sync.dma_start(out=out_tail[:, 0:half], in_=ot[:, 0:half])
                nc.sync.dma_start(out=out_tail[:, half:], in_=ot[:, half:])

        # Drain taper: small per-d blocks at the end so the final store
        # backlog after the last multiply is ~0.4 MB, not ~1.6 MB.
        for d in range(MAX_DISP - NSOLO, MAX_DISP):
            solo(d)
    nc.finalize()
    return nc


def kernel(left: np.ndarray, right: np.ndarray) -> np.ndarray:
    global _NC_CACHE, LAST_RESULTS
    left = np.ascontiguousarray(np.asarray(left, dtype=np.float32))
    right = np.ascontiguousarray(np.asarray(right, dtype=np.float32))
    assert left.shape == (N, C, H, W) and right.shape == (N, C, H, W)

    if _NC_CACHE is None:
        _NC_CACHE = _build_bass()
    nc = _NC_CACHE

    left_flat = np.ascontiguousarray(left.reshape(R, W).astype(np.float16))
    right_flat = np.ascontiguousarray(right.reshape(R, W).astype(np.float16))
    in_maps = [
        {
            "left": left_flat[ROWS * k : ROWS * (k + 1)],
            "right": right_flat[ROWS * k : ROWS * (k + 1)],
        }
        for k in range(NCORES)
    ]

    trace = os.environ.get("COSTVOL_TRACE", "0") == "1"
    kwargs = {}
    if os.environ.get("COSTVOL_TRACE_ALL", "0") == "1":
        kwargs["trace_cores"] = list(range(NCORES))
    res = run_bass_kernel_spmd(
        nc, in_maps, list(range(NCORES)), trace=trace, **kwargs
    )
    LAST_RESULTS = res

    flat = np.zeros((MAX_DISP, R, W), dtype=np.float32)
    for k in range(NCORES):
        big = np.asarray(res.results[k]["out_big"])
        tail = np.asarray(res.results[k]["out_tail"]).reshape(TAIL, MAX_DISP, W)
        r0 = ROWS * k
        for d in SOLOS:
            w = W - d
            blk = big[:, XB[d] : XB[d] + CPP * BW[d]].reshape(128, CPP, BW[d])
            flat[d, r0 + TAIL : r0 + ROWS, d:] = (
                blk[:, :, :w].astype(np.float32).reshape(128 * CPP, w)
            )
        for g in GROUPS:
            wg = W - g
            blk = big[:, XB[g] : XB[g] + G * CPP * wg].reshape(128, G, CPP, wg)
            for i in range(G):
                d = g + i
                w = W - d
                flat[d, r0 + TAIL : r0 + ROWS, d:] = (
                    blk[:, i, :, :w].astype(np.float32).reshape(128 * CPP, w)
                )
        for d in range(MAX_DISP):
            flat[d, r0 : r0 + TAIL, d:] = tail[:, d, : W - d].astype(np.float32)
    vol = flat.reshape(MAX_DISP, N, C, H, W).transpose(1, 2, 0, 3, 4)
    return np.ascontiguousarray(vol)


# revision 34
# speedup vs baseline: 1.0140x; 1.0140x over previous
"""Cost-volume kernel for Trainium2 (Bass/Tile), 8-core SPMD.

volume[n, c, d, h, w] = left[n,c,h,w] * right[n,c,h,w-d]  (0 where w < d)

Sharding: rows (flattened n,c,h = 8704) split as 1088 per core; every core
computes ALL 48 disparities for its rows (shift is along W, so row sharding
needs no halo and inputs are read once).

The kernel is HBM-store bound, so the store stream is minimized two ways:
 - fp16 output (harness gate is rel_err < 2e-2; fp16 product error ~7e-4).
 - packed layout: for disparity d only the ~(W-d) valid products
   packed[d][r, j] = left[r, d+j] * right[r, j] are stored; the host
   scatters them into a zero-filled full volume.
Per core that is ~23.9 MB of DMA vs 52.4 MB for the f32 unpacked baseline.

All multiplies run on DVE (gpsimd tensor_tensor is ~3x slower AND a
concurrent Pool op stalls DVE for its whole duration via SBUF contention;
the ACT engine has no two-tensor op). DVE does ~0.54 ns/elem (2x_1p fp16
mode) plus ~350 ns fixed cost per instruction, so disparities are
processed in GROUPS of 4-8 per instruction using a 4-D access pattern
whose group dim has stride +1 on the left operand (one extra shift per
group member) and stride 0 (broadcast) on the right operand. Group blocks
share a uniform width W-g, so members i>0 carry (d-g) junk columns that
the host ignores. DVE total ~53 us, fully hidden under the ~60-68 us
store stream.

Main chunk: rows [64,1088) as [128 partitions x 8 rows]; per-partition
lines are DRAM-contiguous. Schedule is shaped for the store stream:
d0/d1 ramp in small per-d pieces (first store trigger ~3.3 us after the
first load trigger), the middle runs as G=4/G=8 groups whose stores are
split so per-partition packets stay <=16 KB (bigger packets halve the
per-DMA-engine rate; 12-15 KB packets sustain ~26 B/ns x16 engines), and
d44..47 taper per-d so the post-compute drain is small. The 64-row tail
is ONE flat [64, 48, 240] multiply + two half stores on the SP ring. All
big stores ride the ACT HWDGE ring: alternating rings was tried and
regressed (cross-ring semaphores inflate the DVE window).

Measured (8-core SPMD, core 0): ~78-88 us depending on machine DMA mode
(~416 vs ~347 GB/s effective store bandwidth), vs 148-170 us for the f32
baseline. Fixed overheads: ~7 us engine-init preamble, ~3 us load ramp,
~8.5 us end-of-kernel event-semaphore file reset (257 serial clears
behind the all-DMA barrier; framework-emitted, not avoidable here).
"""

import os

import numpy as np

import bass_rust
import concourse.bacc as bacc
import concourse.mybir as mybir
from concourse.bass_utils import run_bass_kernel_spmd
from concourse.mybir import AluOpType
from concourse.tile import TileContext

N, C, H, W = 2, 32, 136, 240
MAX_DISP = 48
NCORES = 8
R = N * C * H                   # 8704 rows total
ROWS = R // NCORES              # 1088 rows per core
TAIL = 64                       # leftover rows (1088 = 64 + 128*8)
CPP = 8                         # rows per partition in the main chunk
G = 4                           # disparities per grouped DVE instruction
NSOLO = 4                       # leading disparities emitted per-d
LBW = CPP * W + 8               # lb tile width (pad: group reads to 1919+3)
LTW = W + MAX_DISP              # lt tile width (tail reads to 286)

# Per-d blocks: leading d 0..3 (pipeline ramp) and trailing d 44..47
# (drain taper). Groups cover d 4..43: G=4 at the edges, G=8 in the
# middle (fewer per-instruction overheads).
SOLOS = list(range(NSOLO)) + list(range(MAX_DISP - NSOLO, MAX_DISP))
GROUPS = [(4, 4)] + [(g, 8) for g in range(8, 40, 8)] + [(40, 4)]

# Even-rounded block width for the per-d blocks (alignment-safe).
BW = {d: W - d + ((W - d) & 1) for d in SOLOS}

# out_big per-partition column offsets.
XB = {}
_col = 0
for _d in SOLOS:
    XB[_d] = _col
    _col += CPP * BW[_d]
for _g, _gs in GROUPS:
    XB[_g] = _col
    _col += _gs * CPP * (W - _g)
XBTOT = _col

_NC_CACHE = None
LAST_RESULTS = None  # BassKernelResults of the most recent run (for test.py)


def _build_bass():
    # Bacc (not plain Bass): its finalize() runs the compile pipeline incl.
    # generate_event_semaphores, which splits multi-sem waits that walrus
    # rejects ("Too many sync wait commands").
    nc = bacc.Bacc()
    left = nc.dram_tensor("left", [ROWS, W], mybir.dt.float16, kind="ExternalInput")
    right = nc.dram_tensor("right", [ROWS, W], mybir.dt.float16, kind="ExternalInput")
    out_big = nc.dram_tensor(
        "out_big", [128, XBTOT], mybir.dt.float16, kind="ExternalOutput"
    )
    out_tail = nc.dram_tensor(
        "out_tail", [TAIL, MAX_DISP * W], mybir.dt.float16, kind="ExternalOutput"
    )

    with (
        TileContext(nc) as tc,
        tc.tile_pool(name="lpool", bufs=1) as lpool,
        tc.tile_pool(name="rpool", bufs=1) as rpool,
        tc.tile_pool(name="osolo", bufs=4) as osolo,
        tc.tile_pool(name="ogrp", bufs=4) as ogrp,
        tc.tile_pool(name="otail", bufs=1) as otail,
    ):
        lb = lpool.tile([128, LBW], mybir.dt.float16, tag="lbig")
        rb = rpool.tile([128, CPP * W], mybir.dt.float16, tag="rbig")
        lt = lpool.tile([TAIL, LTW], mybir.dt.float16, tag="ltail")
        rt = rpool.tile([TAIL, W], mybir.dt.float16, tag="rtail")

        # Loads split across both HWDGE rings and into q-halves, so the
        # d=0 first-half multiply (which only reads q 0..3) can start after
        # ~1.5 us of load data instead of the full 2.6 us.
        HQ = CPP // 2
        lsrc = left[TAIL:ROWS, :].rearrange("(p q) w -> p q w", p=128)
        rsrc = right[TAIL:ROWS, :].rearrange("(p q) w -> p q w", p=128)
        lbq = lb[:, 0 : CPP * W].rearrange("p (q w) -> p q w", w=W)
        rbq = rb[:].rearrange("p (q w) -> p q w", w=W)
        nc.sync.dma_start(out=lbq[:, 0:2, :], in_=lsrc[:, 0:2, :])
        nc.scalar.dma_start(out=rbq[:, 0:2, :], in_=rsrc[:, 0:2, :])
        nc.sync.dma_start(out=lbq[:, 2:HQ, :], in_=lsrc[:, 2:HQ, :])
        nc.scalar.dma_start(out=rbq[:, 2:HQ, :], in_=rsrc[:, 2:HQ, :])
        nc.sync.dma_start(out=lbq[:, HQ:CPP, :], in_=lsrc[:, HQ:CPP, :])
        nc.scalar.dma_start(out=rbq[:, HQ:CPP, :], in_=rsrc[:, HQ:CPP, :])
        nc.sync.dma_start(out=lt[:, 0:W], in_=left[0:TAIL, :])
        nc.scalar.dma_start(out=rt[:], in_=right[0:TAIL, :])

        rbv = rb[:].rearrange("p (q w) -> p q w", w=W)
        lb_ap = lb[:]

        # Leading disparities per-d: store stream starts after one ~1 us op.
        # Ring selection for big stores (A/B: COSTVOL_DUALRING=1 alternates
        # between both HWDGE rings).
        if os.environ.get("COSTVOL_DUALRING", "0") == "1":
            ring = [nc.scalar, nc.sync]
            nstores = [0]

            def bigstore(dram_ap, sbuf_ap):
                ring[nstores[0] % 2].dma_start(out=dram_ap, in_=sbuf_ap)
                nstores[0] += 1
        else:

            def bigstore(dram_ap, sbuf_ap):
                nc.scalar.dma_start(out=dram_ap, in_=sbuf_ap)

        def solo_part(d, q0, q1):
            bw = BW[d]
            ob = solo_tiles[d]
            nq = q1 - q0
            in0 = bass_rust.AP(
                lb_ap.tensor,
                lb_ap.offset + q0 * W + d,
                [[LBW, 128], [W, nq], [1, bw]],
            )
            nc.vector.tensor_tensor(
                ob[:, q0 * bw : q1 * bw].rearrange("p (q w) -> p q w", w=bw),
                in0,
                rbv[:, q0:q1, 0:bw],
                AluOpType.mult,
            )
            bigstore(
                out_big[:, XB[d] + q0 * bw : XB[d] + q1 * bw],
                ob[:, q0 * bw : q1 * bw],
            )

        solo_tiles = {}

        def solo(d, halves=False):
            solo_tiles[d] = osolo.tile(
                [128, CPP * W], mybir.dt.float16, name="ob_solo"
            )
            for q0, q1 in ([(0, HQ), (HQ, CPP)] if halves else [(0, CPP)]):
                solo_part(d, q0, q1)

        def group(g, gs):
            # One 4-D instruction per gs d's. Left operand group dim strides
            # +1 (one extra shift per member), right operand broadcasts.
            wg = W - g
            ob = ogrp.tile([128, 8 * CPP * (W - CPP)], mybir.dt.float16, name="ob_g")
            in0 = bass_rust.AP(
                lb_ap.tensor,
                lb_ap.offset + g,
                [[LBW, 128], [1, gs], [W, CPP], [1, wg]],
            )
            in1 = rbv[:, :, 0:wg].unsqueeze(1).broadcast_to([128, gs, CPP, wg])
            nc.vector.tensor_tensor(
                ob[:, 0 : gs * CPP * wg].rearrange(
                    "p (i q w) -> p i q w", i=gs, q=CPP
                ),
                in0,
                in1,
                AluOpType.mult,
            )
            # Split stores so per-partition packets stay <=16 KB (bigger
            # packets run at half DMA-engine rate).
            cols = gs * CPP * wg
            nparts = (cols * 2 + 16000) // 16001 if cols * 2 > 16384 else 1
            step = -(-cols // (nparts * CPP * wg)) * CPP * wg if nparts > 1 else cols
            c0 = 0
            while c0 < cols:
                c1 = min(cols, c0 + step)
                bigstore(
                    out_big[:, XB[g] + c0 : XB[g] + c1], ob[:, c0:c1]
                )
                c0 = c1

        def tail_block():
            # Tail: one flat [64, 48, 240] multiply + two half stores on SP
            # (a single 23 KB-per-partition store runs at half engine rate).
            # On DVE: a concurrent Pool op stalls DVE for its whole duration
            # (SBUF contention), so Pool is useless for this.
            ot = otail.tile([TAIL, MAX_DISP * W], mybir.dt.float16)
            t_in0 = bass_rust.AP(
                lt[:].tensor,
                lt[:].offset,
                [[LTW, TAIL], [1, MAX_DISP], [1, W]],
            )
            t_in1 = rt[:].unsqueeze(1).broadcast_to([TAIL, MAX_DISP, W])
            nc.vector.tensor_tensor(
                ot[:].rearrange("p (i w) -> p i w", w=W),
                t_in0,
                t_in1,
                AluOpType.mult,
            )
            half = MAX_DISP * W // 2
            nc.sync.dma_start(out=out_tail[:, 0:half], in_=ot[:, 0:half])
            nc.sync.dma_start(out=out_tail[:, half:], in_=ot[:, half:])

        # Schedule: d0/d1 run as interleaved q-halves (the first-half
        # multiplies only wait on the first-half loads); the first group is
        # hoisted before d2/d3 so its 1.9 MB store bridges the gap while
        # small solos compute; the tail slots in after the first big group.
        solo_tiles[0] = osolo.tile([128, CPP * W], mybir.dt.float16, name="ob_solo")
        solo_tiles[1] = osolo.tile([128, CPP * W], mybir.dt.float16, name="ob_solo")
        solo_part(0, 0, 2)
        solo_part(0, 2, HQ)
        solo_part(1, 0, HQ)
        solo_part(0, HQ, CPP)
        solo_part(1, HQ, CPP)
        group(*GROUPS[0])
        for d in range(2, NSOLO):
            solo(d)
        group(*GROUPS[1])
        tail_block()
        for g, gs in GROUPS[2:]:
            group(g, gs)

        # Drain taper: small per-d blocks at the end so the final store
        # backlog after the last multiply is ~0.4 MB, not ~1.6 MB.
        for d in range(MAX_DISP - NSOLO, MAX_DISP):
            solo(d)
    nc.finalize()
    return nc


def kernel(left: np.ndarray, right: np.ndarray) -> np.ndarray:
    global _NC_CACHE, LAST_RESULTS
    left = np.ascontiguousarray(np.asarray(left, dtype=np.float32))
    right = np.ascontiguousarray(np.asarray(right, dtype=np.float32))
    assert left.shape == (N, C, H, W) and right.shape == (N, C, H, W)

    if _NC_CACHE is None:
        _NC_CACHE = _build_bass()
    nc = _NC_CACHE

    left_flat = np.ascontiguousarray(left.reshape(R, W).astype(np.float16))
    right_flat = np.ascontiguousarray(right.reshape(R, W).astype(np.float16))
    in_maps = [
        {
            "left": left_flat[ROWS * k : ROWS * (k + 1)],
            "right": right_flat[ROWS * k : ROWS * (k + 1)],
        }
        for k in range(NCORES)
    ]

    trace = os.environ.get("COSTVOL_TRACE", "0") == "1"
    kwargs = {}
    if os.environ.get("COSTVOL_TRACE_ALL", "0") == "1":
        kwargs["trace_cores"] = list(range(NCORES))
    res = run_bass_kernel_spmd(
        nc, in_maps, list(range(NCORES)), trace=trace, **kwargs
    )
    LAST_RESULTS = res

    flat = np.zeros((MAX_DISP, R, W), dtype=np.float32)
    for k in range(NCORES):
        big = np.asarray(res.results[k]["out_big"])
        tail = np.asarray(res.results[k]["out_tail"]).reshape(TAIL, MAX_DISP, W)
        r0 = ROWS * k
        for d in SOLOS:
            w = W - d
            blk = big[:, XB[d] : XB[d] + CPP * BW[d]].reshape(128, CPP, BW[d])
            flat[d, r0 + TAIL : r0 + ROWS, d:] = (
                blk[:, :, :w].astype(np.float32).reshape(128 * CPP, w)
            )
        for g, gs in GROUPS:
            wg = W - g
            blk = big[:, XB[g] : XB[g] + gs * CPP * wg].reshape(128, gs, CPP, wg)
            for i in range(gs):
                d = g + i
                w = W - d
                flat[d, r0 + TAIL : r0 + ROWS, d:] = (
                    blk[:, i, :, :w].astype(np.float32).reshape(128 * CPP, w)
                )
        for d in range(MAX_DISP):
            flat[d, r0 : r0 + TAIL, d:] = tail[:, d, : W - d].astype(np.float32)
    vol = flat.reshape(MAX_DISP, N, C, H, W).transpose(1, 2, 0, 3, 4)
    return np.ascontiguousarray(vol)


# revision 39
# speedup vs baseline: 1.1479x; 1.1321x over previous
"""Cost-volume kernel for Trainium2 (Bass/Tile), 8-core SPMD.

volume[n, c, d, h, w] = left[n,c,h,w] * right[n,c,h,w-d]  (0 where w < d)

Sharding: rows (flattened n,c,h = 8704) split as 1088 per core; every core
computes ALL 48 disparities for its rows (shift is along W, so row sharding
needs no halo and inputs are read once).

The kernel is HBM-store bound, so the store stream is minimized two ways:
 - fp16 output (harness gate is rel_err < 2e-2; fp16 product error ~7e-4).
 - packed layout: for disparity d only the ~(W-d) valid products
   packed[d][r, j] = left[r, d+j] * right[r, j] are stored; the host
   scatters them into a zero-filled full volume.
Per core that is ~23.9 MB of DMA vs 52.4 MB for the f32 unpacked baseline.

All multiplies run on DVE (gpsimd tensor_tensor is ~3x slower AND a
concurrent Pool op stalls DVE for its whole duration via SBUF contention;
the ACT engine has no two-tensor op). DVE does ~0.54 ns/elem (2x_1p fp16
mode) plus ~350 ns fixed cost per instruction, so disparities are
processed in GROUPS of 4-8 per instruction using a 4-D access pattern
whose group dim has stride +1 on the left operand (one extra shift per
group member) and stride 0 (broadcast) on the right operand. Group blocks
share a uniform width W-g, so members i>0 carry (d-g) junk columns that
the host ignores. DVE total ~53 us, fully hidden under the ~60-68 us
store stream.

Main chunk: rows [64,1088) as [128 partitions x 8 rows]; per-partition
lines are DRAM-contiguous. Schedule is shaped for the store stream:
d0/d1 ramp in small per-d pieces (first store trigger ~3.3 us after the
first load trigger), the middle runs as G=4/G=8 groups whose stores are
split so per-partition packets stay <=16 KB (bigger packets halve the
per-DMA-engine rate; 12-15 KB packets sustain ~26 B/ns x16 engines), and
d44..47 taper per-d so the post-compute drain is small. The 64-row tail
is ONE flat [64, 48, 240] multiply + two half stores on the SP ring. All
big stores ride the ACT HWDGE ring: alternating rings was tried and
regressed (cross-ring semaphores inflate the DVE window).

Measured (8-core SPMD, core 0): ~78-88 us depending on machine DMA mode
(~416 vs ~347 GB/s effective store bandwidth), vs 148-170 us for the f32
baseline. Fixed overheads: ~7 us engine-init preamble, ~3 us load ramp,
~8.5 us end-of-kernel event-semaphore file reset (257 serial clears
behind the all-DMA barrier; framework-emitted, not avoidable here).
"""

import os

import numpy as np

import bass_rust
import concourse.bacc as bacc
import concourse.mybir as mybir
from concourse.bass_utils import run_bass_kernel_spmd
from concourse.mybir import AluOpType
from concourse.tile import TileContext

N, C, H, W = 2, 32, 136, 240
MAX_DISP = 48
NCORES = 8
R = N * C * H                   # 8704 rows total
ROWS = R // NCORES              # 1088 rows per core
TAIL = 64                       # leftover rows (1088 = 64 + 128*8)
CPP = 8                         # rows per partition in the main chunk
G = 4                           # disparities per grouped DVE instruction
NSOLO = 4                       # leading disparities emitted per-d
LBW = CPP * W + 8               # lb tile width (pad: group reads to 1919+3)
LTW = W + MAX_DISP              # lt tile width (tail reads to 286)

# Per-d blocks: leading d 0..3 (pipeline ramp) and trailing d 44..47
# (drain taper). Groups cover d 4..43: G=4 at the edges, G=8 in the
# middle (fewer per-instruction overheads).
SOLOS = list(range(NSOLO)) + list(range(MAX_DISP - NSOLO, MAX_DISP))
GROUPS = [(4, 4)] + [(g, 8) for g in range(8, 40, 8)] + [(40, 4)]

# Even-rounded block width for the per-d blocks (alignment-safe).
BW = {d: W - d + ((W - d) & 1) for d in SOLOS}

# out_big per-partition column offsets.
XB = {}
_col = 0
for _d in SOLOS:
    XB[_d] = _col
    _col += CPP * BW[_d]
for _g, _gs in GROUPS:
    XB[_g] = _col
    _col += _gs * CPP * (W - _g)
XBTOT = _col

_NC_CACHE = None
LAST_RESULTS = None  # BassKernelResults of the most recent run (for test.py)


def _build_bass():
    # Bacc (not plain Bass): its finalize() runs the compile pipeline incl.
    # generate_event_semaphores, which splits multi-sem waits that walrus
    # rejects ("Too many sync wait commands").
    nc = bacc.Bacc()
    left = nc.dram_tensor("left", [ROWS, W], mybir.dt.float16, kind="ExternalInput")
    right = nc.dram_tensor("right", [ROWS, W], mybir.dt.float16, kind="ExternalInput")
    out_big = nc.dram_tensor(
        "out_big", [128, XBTOT], mybir.dt.float16, kind="ExternalOutput"
    )
    out_tail = nc.dram_tensor(
        "out_tail", [TAIL, MAX_DISP * W], mybir.dt.float16, kind="ExternalOutput"
    )

    # ONE tile pool for everything: each pool's context exit emits a
    # semaphore-clear + all-engine-barrier round into the epilogue, so five
    # pools cost ~4 extra serial barrier rounds after the last byte moves.
    with (
        TileContext(nc) as tc,
        tc.tile_pool(name="pool", bufs=1) as pool,
    ):
        osolo = ogrp = otail = pool
        lb = pool.tile([128, LBW], mybir.dt.float16, tag="lbig")
        rb = pool.tile([128, CPP * W], mybir.dt.float16, tag="rbig")
        lt = pool.tile([TAIL, LTW], mybir.dt.float16, tag="ltail")
        rt = pool.tile([TAIL, W], mybir.dt.float16, tag="rtail")

        # Loads split across both HWDGE rings and into q-halves, so the
        # d=0 first-half multiply (which only reads q 0..3) can start after
        # ~1.5 us of load data instead of the full 2.6 us.
        HQ = CPP // 2
        lsrc = left[TAIL:ROWS, :].rearrange("(p q) w -> p q w", p=128)
        rsrc = right[TAIL:ROWS, :].rearrange("(p q) w -> p q w", p=128)
        lbq = lb[:, 0 : CPP * W].rearrange("p (q w) -> p q w", w=W)
        rbq = rb[:].rearrange("p (q w) -> p q w", w=W)
        nc.sync.dma_start(out=lbq[:, 0:2, :], in_=lsrc[:, 0:2, :])
        nc.scalar.dma_start(out=rbq[:, 0:2, :], in_=rsrc[:, 0:2, :])
        nc.sync.dma_start(out=lbq[:, 2:HQ, :], in_=lsrc[:, 2:HQ, :])
        nc.scalar.dma_start(out=rbq[:, 2:HQ, :], in_=rsrc[:, 2:HQ, :])
        nc.sync.dma_start(out=lbq[:, HQ:CPP, :], in_=lsrc[:, HQ:CPP, :])
        nc.scalar.dma_start(out=rbq[:, HQ:CPP, :], in_=rsrc[:, HQ:CPP, :])
        nc.sync.dma_start(out=lt[:, 0:W], in_=left[0:TAIL, :])
        nc.scalar.dma_start(out=rt[:], in_=right[0:TAIL, :])

        rbv = rb[:].rearrange("p (q w) -> p q w", w=W)
        lb_ap = lb[:]

        # Leading disparities per-d: store stream starts after one ~1 us op.
        # Ring selection for big stores (A/B: COSTVOL_DUALRING=1 alternates
        # between both HWDGE rings).
        if os.environ.get("COSTVOL_DUALRING", "0") == "1":
            ring = [nc.scalar, nc.sync]
            nstores = [0]

            def bigstore(dram_ap, sbuf_ap):
                ring[nstores[0] % 2].dma_start(out=dram_ap, in_=sbuf_ap)
                nstores[0] += 1
        else:

            def bigstore(dram_ap, sbuf_ap):
                nc.scalar.dma_start(out=dram_ap, in_=sbuf_ap)

        def solo_part(d, q0, q1):
            bw = BW[d]
            ob = solo_tiles[d]
            nq = q1 - q0
            in0 = bass_rust.AP(
                lb_ap.tensor,
                lb_ap.offset + q0 * W + d,
                [[LBW, 128], [W, nq], [1, bw]],
            )
            nc.vector.tensor_tensor(
                ob[:, q0 * bw : q1 * bw].rearrange("p (q w) -> p q w", w=bw),
                in0,
                rbv[:, q0:q1, 0:bw],
                AluOpType.mult,
            )
            bigstore(
                out_big[:, XB[d] + q0 * bw : XB[d] + q1 * bw],
                ob[:, q0 * bw : q1 * bw],
            )

        solo_tiles = {}

        def solo(d, halves=False):
            solo_tiles[d] = osolo.tile(
                [128, CPP * W], mybir.dt.float16, name="ob_solo", bufs=4
            )
            for q0, q1 in ([(0, HQ), (HQ, CPP)] if halves else [(0, CPP)]):
                solo_part(d, q0, q1)

        def group(g, gs):
            # One 4-D instruction per gs d's. Left operand group dim strides
            # +1 (one extra shift per member), right operand broadcasts.
            wg = W - g
            ob = ogrp.tile(
                [128, 8 * CPP * (W - CPP)], mybir.dt.float16, name="ob_g", bufs=3
            )
            in0 = bass_rust.AP(
                lb_ap.tensor,
                lb_ap.offset + g,
                [[LBW, 128], [1, gs], [W, CPP], [1, wg]],
            )
            in1 = rbv[:, :, 0:wg].unsqueeze(1).broadcast_to([128, gs, CPP, wg])
            nc.vector.tensor_tensor(
                ob[:, 0 : gs * CPP * wg].rearrange(
                    "p (i q w) -> p i q w", i=gs, q=CPP
                ),
                in0,
                in1,
                AluOpType.mult,
            )
            # Split stores so per-partition packets stay <=16 KB (bigger
            # packets run at half DMA-engine rate).
            cols = gs * CPP * wg
            nparts = (cols * 2 + 16000) // 16001 if cols * 2 > 16384 else 1
            step = -(-cols // (nparts * CPP * wg)) * CPP * wg if nparts > 1 else cols
            c0 = 0
            while c0 < cols:
                c1 = min(cols, c0 + step)
                bigstore(
                    out_big[:, XB[g] + c0 : XB[g] + c1], ob[:, c0:c1]
                )
                c0 = c1

        def tail_block():
            # Tail: one flat [64, 48, 240] multiply + two half stores on SP
            # (a single 23 KB-per-partition store runs at half engine rate).
            # On DVE: a concurrent Pool op stalls DVE for its whole duration
            # (SBUF contention), so Pool is useless for this.
            ot = otail.tile([TAIL, MAX_DISP * W], mybir.dt.float16)
            t_in0 = bass_rust.AP(
                lt[:].tensor,
                lt[:].offset,
                [[LTW, TAIL], [1, MAX_DISP], [1, W]],
            )
            t_in1 = rt[:].unsqueeze(1).broadcast_to([TAIL, MAX_DISP, W])
            nc.vector.tensor_tensor(
                ot[:].rearrange("p (i w) -> p i w", w=W),
                t_in0,
                t_in1,
                AluOpType.mult,
            )
            half = MAX_DISP * W // 2
            nc.sync.dma_start(out=out_tail[:, 0:half], in_=ot[:, 0:half])
            nc.sync.dma_start(out=out_tail[:, half:], in_=ot[:, half:])

        # Schedule: d0/d1 run as interleaved q-halves (the first-half
        # multiplies only wait on the first-half loads); the first group is
        # hoisted before d2/d3 so its 1.9 MB store bridges the gap while
        # small solos compute; the tail slots in after the first big group.
        solo_tiles[0] = osolo.tile(
            [128, CPP * W], mybir.dt.float16, name="ob_solo", bufs=4
        )
        solo_tiles[1] = osolo.tile(
            [128, CPP * W], mybir.dt.float16, name="ob_solo", bufs=4
        )
        solo_part(0, 0, 2)
        solo_part(0, 2, HQ)
        solo_part(1, 0, HQ)
        solo_part(0, HQ, CPP)
        solo_part(1, HQ, CPP)
        group(*GROUPS[0])
        for d in range(2, NSOLO):
            solo(d)
        tail_block()
        for g, gs in GROUPS[1:]:
            group(g, gs)

        # Drain taper: small per-d blocks at the end so the final store
        # backlog after the last multiply is ~0.4 MB, not ~1.6 MB.
        for d in range(MAX_DISP - NSOLO, MAX_DISP):
            solo(d)
    nc.finalize()
    return nc


def kernel(left: np.ndarray, right: np.ndarray) -> np.ndarray:
    global _NC_CACHE, LAST_RESULTS
    left = np.ascontiguousarray(np.asarray(left, dtype=np.float32))
    right = np.ascontiguousarray(np.asarray(right, dtype=np.float32))
    assert left.shape == (N, C, H, W) and right.shape == (N, C, H, W)

    if _NC_CACHE is None:
        _NC_CACHE = _build_bass()
    nc = _NC_CACHE

    left_flat = np.ascontiguousarray(left.reshape(R, W).astype(np.float16))
    right_flat = np.ascontiguousarray(right.reshape(R, W).astype(np.float16))
    in_maps = [
        {
            "left": left_flat[ROWS * k : ROWS * (k + 1)],
            "right": right_flat[ROWS * k : ROWS * (k + 1)],
        }
        for k in range(NCORES)
    ]

    trace = os.environ.get("COSTVOL_TRACE", "0") == "1"
    kwargs = {}
    if os.environ.get("COSTVOL_TRACE_ALL", "0") == "1":
        kwargs["trace_cores"] = list(range(NCORES))
    res = run_bass_kernel_spmd(
        nc, in_maps, list(range(NCORES)), trace=trace, **kwargs
    )
    LAST_RESULTS = res

    flat = np.zeros((MAX_DISP, R, W), dtype=np.float32)
    for k in range(NCORES):
        big = np.asarray(res.results[k]["out_big"])
        tail = np.asarray(res.results[k]["out_tail"]).reshape(TAIL, MAX_DISP, W)
        r0 = ROWS * k
        for d in SOLOS:
            w = W - d
            blk = big[:, XB[d] : XB[d] + CPP * BW[d]].reshape(128, CPP, BW[d])
            flat[d, r0 + TAIL : r0 + ROWS, d:] = (
                blk[:, :, :w].astype(np.float32).reshape(128 * CPP, w)
            )
        for g, gs in GROUPS:
            wg = W - g
            blk = big[:, XB[g] : XB[g] + gs * CPP * wg].reshape(128, gs, CPP, wg)
            for i in range(gs):
                d = g + i
                w = W - d
                flat[d, r0 + TAIL : r0 + ROWS, d:] = (
                    blk[:, i, :, :w].astype(np.float32).reshape(128 * CPP, w)
                )
        for d in range(MAX_DISP):
            flat[d, r0 : r0 + TAIL, d:] = tail[:, d, : W - d].astype(np.float32)
    vol = flat.reshape(MAX_DISP, N, C, H, W).transpose(1, 2, 0, 3, 4)
    return np.ascontiguousarray(vol)
